# revision 1
# baseline (speedup 1.0000x reference)
"""BlockTucker fusion kernel for 8 Trainium2 NeuronCores.

Reference computation (per batch row b):
    h0 = x0 @ W0 + b0; h1 = x1 @ W1 + b1              # [B, 1600]
    per chunk c (20 chunks of 80):
        z[c,o] = sum_{s,t} h0c[s] Wb[c,o,s,t] h1c[t] + bb[c,o]
        z = signsqrt(z); z /= max(||z||_2, 1e-12)
    out = concat(z) @ Wout + bout                      # [B, 3000]

Strategy: pure data parallel over batch (1024 rows/core), bf16 compute.
The bilinear form is computed as an outer-product matmul: for each chunk,
P^T[(s,t), b] = h0[s,b]*h1[t,b] is built feature-major by DMA-replicating
rows of H^T across partitions (8 s-rows x 16 reps / 16 t-rows x 8 reps
per 128-partition tile) and one DVE bf16 multiply per tile; then
z^T[o,b] = sum_{st} WbT[(s,t),o] P^T[(s,t),b] accumulates over 50 k-tiles
in PSUM.
"""

import sys

sys.path.insert(0, "/opt/trn_rl_repo")

from contextlib import ExitStack

import numpy as np
import ml_dtypes

import concourse.bass as bass
import concourse.mybir as mybir
import concourse.tile as tile
from concourse import bacc
from concourse.bass_utils import run_bass_kernel_spmd

BF16 = mybir.dt.bfloat16
F32 = mybir.dt.float32
AF = mybir.ActivationFunctionType

B = 8192
D_IN = 2048
MM = 1600
CHUNKS = 20
CS = 80
D_OUT = 3000
N_CORES = 8
BL = B // N_CORES  # 1024 batch rows per core

K_IN = D_IN // 128  # 16 k-tiles for projections
MT_H = 13  # m-tiles for H (1600 -> 12x128 + 64)
MM_PAD = MT_H * 128  # 1664
KT_BIL = 50  # k-tiles per chunk for bilinear (6400/128)
MT_O = 24  # m-tiles for out (3000 -> 23x128 + 56)
D_OUT_PAD = MT_O * 128  # 3072
NH = BL // 512  # 2 free-dim halves of 512


def _h_row_segments(mm0, nrows):
    """Split H rows [mm0, mm0+nrows) at 128-partition boundaries.

    Yields (row_off_in_block, p0, kt, n) for each contiguous segment.
    """
    segs = []
    a = mm0
    while a < mm0 + nrows:
        p0 = a % 128
        kt = a // 128
        n = min(128 - p0, mm0 + nrows - a)
        segs.append((a - mm0, p0, kt, n))
        a += n
    return segs


def _rep_dma(nc, dst, src_h, mm0, nrows, reps):
    """dst[128, BL] <- partition p gets H row (mm0 + p//reps).

    mm0 is reps-aligned-enough that the nrows-run never crosses a
    128-partition boundary of the packed H layout (8- and 16-row runs at
    8/16-aligned offsets).
    """
    p0, kt = mm0 % 128, mm0 // 128
    assert p0 + nrows <= 128
    sl = src_h[p0 : p0 + nrows, kt, :]  # [nrows, BL]
    rep = sl.unsqueeze(1).broadcast_to([nrows, reps, BL])
    nc.sync.dma_start(out=dst, in_=rep)


def _dup_dma(nc, dst, src_dram, mm0, nrows, reps):
    """dst[128, BL] <- partition p gets H row (mm0 + p % nrows).

    src_dram must be a DRAM mirror of the packed H layout: dim0 stride-0
    (the replication) is only legal on DRAM-side APs.
    """
    p0, kt = mm0 % 128, mm0 // 128
    assert p0 + nrows <= 128
    sl = src_dram[p0 : p0 + nrows, kt, :]  # [nrows, BL]
    rep = sl.unsqueeze(0).broadcast_to([reps, nrows, BL])
    nc.sync.dma_start(out=dst, in_=rep)


def build_program():
    nc = bacc.Bacc("TRN2", target_bir_lowering=False, debug=False)

    # DRAM parameters (per-core shards / replicated weights)
    x0T = nc.dram_tensor("x0T", [D_IN, BL], F32, kind="ExternalInput").ap()
    x1T = nc.dram_tensor("x1T", [D_IN, BL], F32, kind="ExternalInput").ap()
    w0 = nc.dram_tensor("w0", [D_IN, MM], BF16, kind="ExternalInput").ap()
    w1 = nc.dram_tensor("w1", [D_IN, MM], BF16, kind="ExternalInput").ap()
    wbp = nc.dram_tensor("wbp", [CHUNKS, KT_BIL, 128, CS], BF16, kind="ExternalInput").ap()
    wout = nc.dram_tensor("wout", [MM_PAD, D_OUT], BF16, kind="ExternalInput").ap()
    b0c = nc.dram_tensor("b0c", [128, MT_H], F32, kind="ExternalInput").ap()
    b1c = nc.dram_tensor("b1c", [128, MT_H], F32, kind="ExternalInput").ap()
    bbT = nc.dram_tensor("bbT", [CS, CHUNKS], F32, kind="ExternalInput").ap()
    boutc = nc.dram_tensor("boutc", [128, MT_O], F32, kind="ExternalInput").ap()
    outT = nc.dram_tensor("outT", [D_OUT, BL], F32, kind="ExternalOutput").ap()
    rn_dram = nc.dram_tensor("rn_dram", [CHUNKS, BL], F32).ap()
    h1_dram = nc.dram_tensor("h1_dram", [128, MT_H, BL], BF16).ap()

    with tile.TileContext(nc) as tc:
        _emit(
            tc, nc, x0T, x1T, w0, w1, wbp, wout, b0c, b1c, bbT, boutc, outT,
            rn_dram, h1_dram,
        )
    nc.compile()
    return nc


def _emit(
    tc, nc, x0T, x1T, w0, w1, wbp, wout, b0c, b1c, bbT, boutc, outT, rn_dram, h1_dram
):
    ctx = ExitStack()
    with ctx:
        singles = ctx.enter_context(tc.tile_pool(name="singles", bufs=1))
        hpool = ctx.enter_context(tc.tile_pool(name="hpool", bufs=1))
        mm_psum = ctx.enter_context(tc.tile_pool(name="mm_psum", bufs=2, space="PSUM"))
        zpsum_pool = ctx.enter_context(tc.tile_pool(name="zpsum", bufs=2, space="PSUM"))
        nsq_psum = ctx.enter_context(tc.tile_pool(name="nsq_psum", bufs=1, space="PSUM"))

        # constants / biases
        b0s = singles.tile([128, MT_H], F32)
        nc.sync.dma_start(out=b0s, in_=b0c)
        b1s = singles.tile([128, MT_H], F32)
        nc.sync.dma_start(out=b1s, in_=b1c)
        bbs = singles.tile([CS, CHUNKS], F32)
        nc.sync.dma_start(out=bbs, in_=bbT)
        bouts = singles.tile([128, MT_O], F32)
        nc.sync.dma_start(out=bouts, in_=boutc)
        ones80 = singles.tile([CS, 1], BF16)
        nc.vector.memset(ones80, 1.0)

        # Z (normalized, repacked) for the final matmul: rows = c*80+o, padded
        zbig = singles.tile([128, MT_H, BL], BF16)
        nc.vector.memset(zbig[64:128, MT_H - 1, :], 0.0)

        HALF = CHUNKS // 5
        nsq_half, nrm_half = [], []
        for i in range(CHUNKS // HALF):
            nsq_h = singles.tile([HALF, BL], F32, tag=f"nsq{i}")
            nsq_half.append(nsq_h)
            nrm_h = singles.tile([HALF, BL], F32, tag=f"nrm{i}")
            nrm_half.append(nrm_h)
        rn_half = nsq_half  # reciprocal overwrites nsq in place

        h0s = hpool.tile([128, MT_H, BL], BF16, tag="h0")

        # ---- Phase 1: projections. h0 -> resident SBUF tile (feature-major
        # bf16); h1 -> per-m-tile evac straight to its DRAM mirror (read back
        # by the interleaved-replication DMAs). ----
        with tc.tile_pool(name="xpool", bufs=2) as xpool, tc.tile_pool(
            name="wproj", bufs=3
        ) as wproj, tc.tile_pool(name="h1e", bufs=3) as h1e_pool:
            for inp_idx, (xT, wdram, bias_s) in enumerate(
                ((x0T, w0, b0s), (x1T, w1, b1s))
            ):
                xs = xpool.tile([128, K_IN, BL], BF16, tag="xs")
                xr = xT.rearrange("(kt p) b -> p kt b", p=128)
                for kt4 in range(0, K_IN, 4):
                    nc.gpsimd.dma_start(
                        out=xs[:, kt4 : kt4 + 4, :], in_=xr[:, kt4 : kt4 + 4, :]
                    )
                for mt in range(MT_H):
                    m0 = mt * 128
                    mw = min(128, MM - m0)
                    wt = wproj.tile([128, K_IN, 128], BF16, tag="wt")
                    nc.sync.dma_start(
                        out=wt[:, :, :mw],
                        in_=wdram[:, m0 : m0 + mw].rearrange(
                            "(kt p) m -> p kt m", p=128
                        ),
                    )
                    h1ev = None
                    if inp_idx == 1:
                        h1ev = h1e_pool.tile([128, BL], BF16, tag="h1ev")
                    for h in range(NH):
                        ps = mm_psum.tile([128, 512], F32, tag="mmps")
                        for kt in range(K_IN):
                            nc.tensor.matmul(
                                out=ps[:mw, :],
                                lhsT=wt[:, kt, :mw],
                                rhs=xs[:, kt, h * 512 : (h + 1) * 512],
                                start=(kt == 0),
                                stop=(kt == K_IN - 1),
                            )
                        dst = (
                            h0s[:mw, mt, h * 512 : (h + 1) * 512]
                            if inp_idx == 0
                            else h1ev[:mw, h * 512 : (h + 1) * 512]
                        )
                        nc.scalar.activation(
                            out=dst,
                            in_=ps[:mw, :],
                            func=AF.Identity,
                            bias=bias_s[:mw, mt : mt + 1],
                            scale=1.0,
                        )
                    if inp_idx == 1:
                        nc.sync.dma_start(
                            out=h1_dram[:mw, mt, :], in_=h1ev[:mw, :]
                        )

        # ---- Phases 2+3: bilinear per chunk, then norms + repack ----
        with ExitStack() as p23:
            dup_pool = p23.enter_context(tc.tile_pool(name="dup", bufs=2))
            rep_pool = p23.enter_context(tc.tile_pool(name="rep", bufs=3))
            p_pool = p23.enter_context(tc.tile_pool(name="ppool", bufs=2))
            wb_pool = p23.enter_context(tc.tile_pool(name="wbpool", bufs=4))
            post_pool = p23.enter_context(tc.tile_pool(name="post", bufs=2))
            zs_pool = p23.enter_context(tc.tile_pool(name="zs", bufs=CHUNKS))
            nsq1_pool = p23.enter_context(tc.tile_pool(name="nsq1", bufs=1))
            rnb_pool = p23.enter_context(tc.tile_pool(name="rnb", bufs=2))
            zn_pool = p23.enter_context(tc.tile_pool(name="zn", bufs=2))

            zs_tiles = []
            for c in range(CHUNKS):
                h1dup = dup_pool.tile([128, 5, BL], BF16, tag="h1dup")
                for j in range(5):
                    _dup_dma(nc, h1dup[:, j, :], h1_dram, 80 * c + 16 * j, 16, 8)
                zps = zpsum_pool.tile([CS, BL], F32, tag="zps")
                for i in range(10):
                    h0rep = rep_pool.tile([128, BL], BF16, tag="h0rep")
                    _rep_dma(nc, h0rep, h0s, 80 * c + 8 * i, 8, 16)
                    wbt = wb_pool.tile([128, 5, CS], BF16, tag="wbt")
                    nc.sync.dma_start(
                        out=wbt,
                        in_=wbp[c, 5 * i : 5 * i + 5].rearrange("j p o -> p j o"),
                    )
                    pt5 = p_pool.tile([128, 5, BL], BF16, tag="pt5")
                    nc.vector.tensor_mul(
                        pt5,
                        h0rep.unsqueeze(1).broadcast_to([128, 5, BL]),
                        h1dup,
                    )
                    for j in range(5):
                        kt = 5 * i + j
                        for h in range(NH):
                            nc.tensor.matmul(
                                out=zps[:, h * 512 : (h + 1) * 512],
                                lhsT=wbt[:, j, :],
                                rhs=pt5[:, j, h * 512 : (h + 1) * 512],
                                start=(kt == 0),
                                stop=(kt == KT_BIL - 1),
                            )
                # post: a = |z+bb|, g = sign(z+bb), s = sqrt(a), zs = s*g
                av = post_pool.tile([CS, BL], BF16, tag="av")
                nc.scalar.activation(
                    out=av, in_=zps, func=AF.Abs, bias=bbs[:, c : c + 1], scale=1.0
                )
                gv = post_pool.tile([CS, BL], BF16, tag="gv")
                nc.scalar.activation(
                    out=gv, in_=zps, func=AF.Sign, bias=bbs[:, c : c + 1], scale=1.0
                )
                sv = post_pool.tile([CS, BL], BF16, tag="sv")
                nc.scalar.activation(out=sv, in_=av, func=AF.Sqrt)
                zst = zs_pool.tile([CS, BL], BF16, tag="zst")
                nc.gpsimd.tensor_mul(zst, sv, gv)
                zs_tiles.append(zst)
                # nsq[b] = sum_o |z+bb| ( = ||signsqrt(z)||^2 )
                nps = nsq_psum.tile([1, BL], F32, tag="nps")
                for h in range(NH):
                    nc.tensor.matmul(
                        out=nps[:, h * 512 : (h + 1) * 512],
                        lhsT=ones80,
                        rhs=av[:, h * 512 : (h + 1) * 512],
                        start=True,
                        stop=True,
                    )
                nsq1 = nsq1_pool.tile([1, BL], F32, tag="nsq1")
                nc.scalar.copy(nsq1, nps)
                half, hc = divmod(c, HALF)
                nc.sync.dma_start(out=nsq_half[half][hc : hc + 1, :], in_=nsq1)

            # ---- Phase 3: norms + apply + repack (two batches to overlap
            # the first batch with the second half of the bilinear) ----
            for half in range(CHUNKS // HALF):
                lo, hi = half * HALF, (half + 1) * HALF
                nc.scalar.activation(
                    out=nrm_half[half], in_=nsq_half[half], func=AF.Sqrt
                )
                nc.vector.tensor_scalar_max(nrm_half[half], nrm_half[half], 1e-12)
                nc.vector.reciprocal(rn_half[half], nrm_half[half])
                nc.sync.dma_start(out=rn_dram[lo:hi, :], in_=rn_half[half])
                for c in range(lo, hi):
                    rnb = rnb_pool.tile([CS, BL], BF16, tag="rnb")
                    nc.gpsimd.dma_start(
                        out=rnb, in_=rn_dram[c : c + 1, :].partition_broadcast(CS)
                    )
                    zn = zn_pool.tile([CS, BL], BF16, tag="zn")
                    nc.gpsimd.tensor_mul(zn, zs_tiles[c], rnb)
                    for off, p0, kt, n in _h_row_segments(80 * c, CS):
                        nc.sync.dma_start(
                            out=zbig[p0 : p0 + n, kt, :], in_=zn[off : off + n, :]
                        )

        # ---- Phase 4: out^T = Wout^T-style matmul + bout ----
        with tc.tile_pool(name="wo", bufs=3) as wo_pool, tc.tile_pool(
            name="opool", bufs=2
        ) as o_pool:
            for mt in range(MT_O):
                m0 = mt * 128
                mw = min(128, D_OUT - m0)
                wot = wo_pool.tile([128, MT_H, 128], BF16, tag="wot")
                nc.sync.dma_start(
                    out=wot[:, :, :mw],
                    in_=wout[:, m0 : m0 + mw].rearrange("(kt p) m -> p kt m", p=128),
                )
                for h in range(NH):
                    ps = mm_psum.tile([128, 512], F32, tag="mmps")
                    for kt in range(MT_H):
                        nc.tensor.matmul(
                            out=ps[:mw, :],
                            lhsT=wot[:, kt, :mw],
                            rhs=zbig[:, kt, h * 512 : (h + 1) * 512],
                            start=(kt == 0),
                            stop=(kt == MT_H - 1),
                        )
                    ot = o_pool.tile([128, 512], F32, tag="ot")
                    nc.scalar.activation(
                        out=ot[:mw, :],
                        in_=ps[:mw, :],
                        func=AF.Identity,
                        bias=bouts[:mw, mt : mt + 1],
                        scale=1.0,
                    )
                    nc.sync.dma_start(
                        out=outT[m0 : m0 + mw, h * 512 : (h + 1) * 512],
                        in_=ot[:mw, :],
                    )


_PROGRAM = None


def _get_program():
    global _PROGRAM
    if _PROGRAM is None:
        _PROGRAM = build_program()
    return _PROGRAM


def prep_weights(W0, b0, W1, b1, Wb, bb, Wout, bout):
    bf = ml_dtypes.bfloat16
    w0 = np.ascontiguousarray(W0, dtype=bf)
    w1 = np.ascontiguousarray(W1, dtype=bf)
    # Wbp[c, 5i+j, p, o] = Wb[c, o, 8i + p//16, 16j + p%16]
    p = np.arange(128)
    wbp = np.empty((CHUNKS, KT_BIL, 128, CS), dtype=bf)
    for i in range(10):
        s_idx = 8 * i + p // 16
        for j in range(5):
            t_idx = 16 * j + p % 16
            wbp[:, 5 * i + j] = Wb[:, :, s_idx, t_idx].transpose(0, 2, 1)
    woutp = np.zeros((MM_PAD, D_OUT), dtype=bf)
    woutp[:MM] = Wout
    b0p = np.zeros(MM_PAD, np.float32)
    b0p[:MM] = b0
    b0c = np.ascontiguousarray(b0p.reshape(MT_H, 128).T)
    b1p = np.zeros(MM_PAD, np.float32)
    b1p[:MM] = b1
    b1c = np.ascontiguousarray(b1p.reshape(MT_H, 128).T)
    bbT = np.ascontiguousarray(np.asarray(bb, np.float32).T)
    boutp = np.zeros(D_OUT_PAD, np.float32)
    boutp[:D_OUT] = bout
    boutc = np.ascontiguousarray(boutp.reshape(MT_O, 128).T)
    return dict(
        w0=w0, w1=w1, wbp=wbp, wout=woutp, b0c=b0c, b1c=b1c, bbT=bbT, boutc=boutc
    )


def make_in_maps(x0, x1, weights):
    x0T = np.ascontiguousarray(np.asarray(x0, np.float32).T)
    x1T = np.ascontiguousarray(np.asarray(x1, np.float32).T)
    in_maps = []
    for r in range(N_CORES):
        sl = slice(r * BL, (r + 1) * BL)
        m = dict(weights)
        m["x0T"] = np.ascontiguousarray(x0T[:, sl])
        m["x1T"] = np.ascontiguousarray(x1T[:, sl])
        in_maps.append(m)
    return in_maps


def run(x0, x1, weights, **kwargs):
    nc = _get_program()
    in_maps = make_in_maps(x0, x1, weights)
    res = run_bass_kernel_spmd(nc, in_maps, core_ids=list(range(N_CORES)), **kwargs)
    out = np.empty((B, D_OUT), np.float32)
    for r in range(N_CORES):
        out[r * BL : (r + 1) * BL, :] = res.results[r]["outT"].T
    return out, res


def kernel(x0, x1, W0, b0, W1, b1, Wb, bb, Wout, bout):
    weights = prep_weights(W0, b0, W1, b1, Wb, bb, Wout, bout)
    out, _ = run(x0, x1, weights)
    return out


# ---- timed runner (no NTFF hook in this container: wall-clock the PJRT
# executable with device-resident inputs, minus dispatch overhead) ----

def _make_sharded_callable(nc, in_maps):
    import jax
    import numpy as _np
    from jax.sharding import Mesh, PartitionSpec, NamedSharding
    from jax.experimental.shard_map import shard_map
    from concourse import bass2jax as b2j
    from concourse import mybir as _mybir

    b2j.install_neuronx_cc_hook()
    n_cores = len(in_maps)
    partition_name = nc.partition_id_tensor.name if nc.partition_id_tensor else None
    in_names, out_names, out_avals, zero_outs = [], [], [], []
    for alloc in nc.m.functions[0].allocations:
        if not isinstance(alloc, _mybir.MemoryLocationSet):
            continue
        name = alloc.memorylocations[0].name
        if alloc.kind == "ExternalInput":
            if name != partition_name:
                in_names.append(name)
        elif alloc.kind == "ExternalOutput":
            shape = tuple(alloc.tensor_shape)
            dtype = _mybir.dt.np(alloc.dtype)
            out_names.append(name)
            out_avals.append(jax.core.ShapedArray(shape, dtype))
            zero_outs.append(_np.zeros(shape, dtype))
    n_params = len(in_names)
    in_names_all = list(in_names) + list(out_names)
    if partition_name is not None:
        in_names_all.append(partition_name)

    def _body(*args):
        operands = list(args)
        if partition_name is not None:
            operands.append(b2j.partition_id_tensor())
        outs = b2j._bass_exec_p.bind(
            *operands,
            out_avals=tuple(out_avals),
            in_names=tuple(in_names_all),
            out_names=tuple(out_names),
            lowering_input_output_aliases=(),
            sim_require_finite=True,
            sim_require_nnan=True,
            nc=nc,
        )
        return tuple(outs)

    devices = jax.devices()[:n_cores]
    mesh = Mesh(_np.asarray(devices), ("core",))
    spec = PartitionSpec("core")
    in_specs = (spec,) * (n_params + len(out_names))
    out_specs = (spec,) * len(out_names)
    n_outs = len(out_names)
    donate = tuple(range(n_params, n_params + n_outs))
    sharded = jax.jit(
        shard_map(_body, mesh=mesh, in_specs=in_specs, out_specs=out_specs,
                  check_rep=False),
        keep_unused=True,
        donate_argnums=donate,
    )
    sh = NamedSharding(mesh, spec)
    concat_in = [
        jax.device_put(
            _np.concatenate([_np.asarray(in_maps[c][n]) for c in range(n_cores)], 0), sh
        )
        for n in in_names
    ]
    state = {"outs": None}

    def _fresh_zeros():
        return [
            jax.device_put(_np.zeros((n_cores * z.shape[0], *z.shape[1:]), z.dtype), sh)
            for z in zero_outs
        ]

    def call():
        # outputs are donated back in as the next call's output seeds; the
        # kernel fully overwrites every output, so contents don't matter
        seeds = state["outs"] if state["outs"] is not None else _fresh_zeros()
        outs = sharded(*concat_in, *seeds)
        state["outs"] = list(outs)
        return outs
    return call, out_names, out_avals


def bench(x0, x1, weights, iters=30):
    """Returns (out, per_iter_seconds_list)."""
    import jax, time
    nc = _get_program()
    in_maps = make_in_maps(x0, x1, weights)
    call, out_names, out_avals = _make_sharded_callable(nc, in_maps)
    res = call()
    jax.block_until_ready(res)
    times = []
    for _ in range(iters):
        t0 = time.perf_counter_ns()
        r = call()
        jax.block_until_ready(r)
        times.append((time.perf_counter_ns() - t0))
    out_arr = np.asarray(res[out_names.index("outT")]).reshape(N_CORES, D_OUT, BL)
    out = np.empty((B, D_OUT), np.float32)
    for r_ in range(N_CORES):
        out[r_ * BL : (r_ + 1) * BL, :] = out_arr[r_].T
    return out, times


def bench_overhead(iters=30):
    """Dispatch overhead baseline: trivial 1-DMA kernel through same path."""
    import jax, time
    global _TINY
    try:
        nc = _TINY
    except NameError:
        nc = None
    if nc is None:
        nc = bacc.Bacc("TRN2", target_bir_lowering=False, debug=False)
        a = nc.dram_tensor("a", [128, 16], F32, kind="ExternalInput").ap()
        o = nc.dram_tensor("o", [128, 16], F32, kind="ExternalOutput").ap()
        with tile.TileContext(nc) as tc:
            with tc.tile_pool(name="p", bufs=1) as pool:
                t = pool.tile([128, 16], F32)
                nc.sync.dma_start(out=t, in_=a)
                nc.sync.dma_start(out=o, in_=t)
        nc.compile()
        _TINY = nc
    in_maps = [dict(a=np.zeros((128, 16), np.float32)) for _ in range(N_CORES)]
    call, _, _ = _make_sharded_callable(nc, in_maps)
    res = call()
    jax.block_until_ready(res)
    times = []
    for _ in range(iters):
        t0 = time.perf_counter_ns()
        r = call()
        jax.block_until_ready(r)
        times.append(time.perf_counter_ns() - t0)
    return times


def bench_async(x0, x1, weights, iters=50):
    """Amortized per-iter time: N async dispatches, single block at the end."""
    import jax, time
    nc = _get_program()
    in_maps = make_in_maps(x0, x1, weights)
    call, out_names, out_avals = _make_sharded_callable(nc, in_maps)
    res = call()
    jax.block_until_ready(res)
    # pipeline warmup
    rs = [call() for _ in range(5)]
    jax.block_until_ready(rs)
    t0 = time.perf_counter_ns()
    rs = [call() for _ in range(iters)]
    jax.block_until_ready(rs)
    dt = time.perf_counter_ns() - t0
    out_arr = np.asarray(res[out_names.index("outT")]).reshape(N_CORES, D_OUT, BL)
    out = np.empty((B, D_OUT), np.float32)
    for r_ in range(N_CORES):
        out[r_ * BL : (r_ + 1) * BL, :] = out_arr[r_].T
    return out, dt / iters


def bench_async_overhead(iters=50):
    import jax, time
    global _TINY2
    try:
        nc = _TINY2
    except NameError:
        nc = None
    if nc is None:
        nc = bacc.Bacc("TRN2", target_bir_lowering=False, debug=False)
        a = nc.dram_tensor("a", [128, 16], F32, kind="ExternalInput").ap()
        o = nc.dram_tensor("o", [128, 16], F32, kind="ExternalOutput").ap()
        with tile.TileContext(nc) as tc:
            with tc.tile_pool(name="p", bufs=1) as pool:
                t = pool.tile([128, 16], F32)
                nc.sync.dma_start(out=t, in_=a)
                nc.sync.dma_start(out=o, in_=t)
        nc.compile()
        _TINY2 = nc
    in_maps = [dict(a=np.zeros((128, 16), np.float32)) for _ in range(N_CORES)]
    call, _, _ = _make_sharded_callable(nc, in_maps)
    import jax as _j
    _j.block_until_ready(call())
    rs = [call() for _ in range(5)]
    _j.block_until_ready(rs)
    import time as _t
    t0 = _t.perf_counter_ns()
    rs = [call() for _ in range(iters)]
    _j.block_until_ready(rs)
    return (_t.perf_counter_ns() - t0) / iters


def _make_tiny_callable():
    global _TINY3
    try:
        nc = _TINY3
    except NameError:
        nc = None
    if nc is None:
        nc = bacc.Bacc("TRN2", target_bir_lowering=False, debug=False)
        a = nc.dram_tensor("a", [128, 16], F32, kind="ExternalInput").ap()
        o = nc.dram_tensor("o", [128, 16], F32, kind="ExternalOutput").ap()
        with tile.TileContext(nc) as tc:
            with tc.tile_pool(name="p", bufs=1) as pool:
                t = pool.tile([128, 16], F32)
                nc.sync.dma_start(out=t, in_=a)
                nc.sync.dma_start(out=o, in_=t)
        nc.compile()
        _TINY3 = nc
    in_maps = [dict(a=np.zeros((128, 16), np.float32)) for _ in range(N_CORES)]
    call, _, _ = _make_sharded_callable(nc, in_maps)
    return call

